# revision 16
# baseline (speedup 1.0000x reference)
"""Trainium2 Bass kernel for nn_Concat_84653805404632.

Reference computation: x is [70, 128, 512] f32; rows 0..19 are supports
(ns_all = n_class*n_support = 20), rows 20..69 are queries (nq_all = 50).
Output [1000, 128, 1024] where out[q*20+s] = concat(sup[s], qry[q], axis=-1).

Pure data movement (memory regime). Sharding: the (query, support) pair grid
[50 x 20] is split as (2 query-halves) x (4 support-fifths) -> 8 cores, each
producing exactly 125 output rows (64 MB) with an identical SPMD access
pattern.

Per core: the 5 support tiles are DMA-loaded directly into the sup columns of
two interleaved "image" buffers in SBUF; the VectorEngine broadcasts each
query tile into the qry columns (SBUF engine ports are separate from the DMA
AXI ports, so this overlaps the writes for free); each query then leaves as
ONE 2.62 MB write DMA whose descriptors are full 4 KB rows — the largest
descriptor this output layout allows, which keeps all 16 SDMA engines at
their ~25 B/ns per-descriptor rate (~400-420 GB/s per core, the 16-engine
descriptor-processing ceiling). Writes double-buffer against the DVE copies;
the load order (sup image 0, first query chunk, sup image 1, remaining
chunks) lets the first write start as early as possible.

Measured on 8 trn2 cores: 195-221 us NEFF exec depending on device state
(the chip oscillates between a ~420 GB/s/core and a ~350 GB/s/core HBM/DMA
regime run-to-run), rel err 0. Within a fixed state this layout beat the
per-query-load variant by ~4% by moving 1.3 MB fewer load bytes per core.
"""

import os
import sys

import numpy as np

for _p in ("/opt/trn_rl_repo", "/root/.axon_site/_ro/trn_rl_repo"):
    if os.path.isdir(_p) and _p not in sys.path:
        sys.path.insert(0, _p)

import concourse.bass as bass
import concourse.mybir as mybir
from concourse.bass_utils import run_bass_kernel_spmd

NS_ALL = 20  # n_class * n_support
NQ_ALL = 50  # n_class * n_query
D = 128
F = 512
QH = 25  # queries per core  (NQ_ALL / 2)
SF = 5  # supports per core (NS_ALL / 4)
QCH = 5  # query tiles per load chunk
N_CORES = 8

_NC_CACHE = None


def _build_nc():
    nc = bass.Bass()
    sup = nc.declare_dram_parameter("sup", [D, SF, F], mybir.dt.float32, isOutput=False)
    qry = nc.declare_dram_parameter("qry", [D, QH, F], mybir.dt.float32, isOutput=False)
    out = nc.declare_dram_parameter(
        "out", [D, QH * SF, 2 * F], mybir.dt.float32, isOutput=True
    )

    with (
        nc.sbuf_tensor([D, QH * F], mybir.dt.float32) as qry_t,
        nc.sbuf_tensor([D, SF * F], mybir.dt.float32) as sup_t,
        nc.sbuf_tensor([D, SF * 2 * F], mybir.dt.float32) as img0,
        nc.sbuf_tensor([D, SF * 2 * F], mybir.dt.float32) as img1,
        nc.semaphore("sup_sem") as sup_sem,
        nc.semaphore("qry_sem0") as qry_sem0,
        nc.semaphore("qry_sem1") as qry_sem1,
        nc.semaphore("qry_sem2") as qry_sem2,
        nc.semaphore("qry_sem3") as qry_sem3,
        nc.semaphore("qry_sem4") as qry_sem4,
        nc.semaphore("dve_sem") as dve_sem,
        nc.semaphore("out_sem0") as out_sem0,
        nc.semaphore("out_sem1") as out_sem1,
        nc.Block() as block,
    ):
        imgs = [img0, img1]
        qry_sems = [qry_sem0, qry_sem1, qry_sem2, qry_sem3, qry_sem4]
        out_sems = [out_sem0, out_sem1]

        def img_view(b):
            return imgs[b][:].rearrange("p (s f2) -> p s f2", f2=2 * F)

        @block.sync
        def _(sync):
            sync.dma_start(sup_t[:], sup[:]).then_inc(sup_sem, 16)
            for c in range(QH // QCH):
                sync.dma_start(
                    qry_t[:, QCH * F * c : QCH * F * (c + 1)],
                    qry[:, QCH * c : QCH * (c + 1), :],
                ).then_inc(qry_sems[c], 16)

        @block.vector
        def _(vector):
            sup_v = sup_t[:].rearrange("p (s f) -> p s f", f=F)
            vector.wait_ge(sup_sem, 16)
            vector.tensor_copy(img_view(0)[:, :, 0:F], sup_v).then_inc(dve_sem, 1)

            def qcopy(q):
                vector.wait_ge(qry_sems[q // QCH], 16)
                if q >= 2:
                    vector.wait_ge(out_sems[q % 2], 16 * (q // 2))
                dst = img_view(q % 2)[:, :, F : 2 * F]
                src = (
                    qry_t[:, F * q : F * (q + 1)]
                    .unsqueeze(1)
                    .broadcast_to([D, SF, F])
                )
                vector.tensor_copy(dst, src).then_inc(dve_sem, 1)

            qcopy(0)
            vector.tensor_copy(img_view(1)[:, :, 0:F], sup_v).then_inc(dve_sem, 1)
            for q in range(1, QH):
                qcopy(q)

        @block.scalar
        def _(scalar):
            for q in range(QH):
                scalar.wait_ge(dve_sem, 2 if q == 0 else q + 3)
                # p-major dst: rows 5q..5q+5 of every partition, contiguous
                # 20KB per partition; 4KB descriptors via max_dma_last_dim
                dst = out[:, SF * q : SF * (q + 1), :]
                scalar.dma_start(
                    dst, imgs[q % 2][:], max_dma_last_dim=1024
                ).then_inc(out_sems[q % 2], 16)
            scalar.wait_ge(out_sem0, 16 * ((QH + 1) // 2))
            scalar.wait_ge(out_sem1, 16 * (QH // 2))

    return nc


def _get_nc():
    global _NC_CACHE
    if _NC_CACHE is None:
        _NC_CACHE = _build_nc()
    return _NC_CACHE


def kernel(**inputs) -> np.ndarray:
    x = np.ascontiguousarray(np.asarray(inputs["x"], dtype=np.float32))
    assert x.shape == (NS_ALL + NQ_ALL, D, F), x.shape

    sup_all = x[:NS_ALL]
    qry_all = x[NS_ALL:]

    in_maps = []
    for k in range(N_CORES):
        h, f = divmod(k, 4)
        in_maps.append(
            {
                # transposed to [D, n, F] so load DMAs are contiguous on both
                # sides (4KB descriptors via max_dma_last_dim)
                "sup": np.ascontiguousarray(
                    sup_all[SF * f : SF * (f + 1)].transpose(1, 0, 2)
                ),
                "qry": np.ascontiguousarray(
                    qry_all[QH * h : QH * (h + 1)].transpose(1, 0, 2)
                ),
            }
        )

    nc = _get_nc()
    res = run_bass_kernel_spmd(nc, in_maps, core_ids=list(range(N_CORES)))

    full = np.empty((NQ_ALL, NS_ALL, D, 2 * F), dtype=np.float32)
    for k in range(N_CORES):
        h, f = divmod(k, 4)
        out_k = (
            np.asarray(res.results[k]["out"])
            .transpose(1, 0, 2)
            .reshape(QH, SF, D, 2 * F)
        )
        full[QH * h : QH * (h + 1), SF * f : SF * (f + 1)] = out_k
    return full.reshape(NQ_ALL * NS_ALL, D, 2 * F)


# revision 17
# speedup vs baseline: 1.0025x; 1.0025x over previous
"""Trainium2 Bass kernel for nn_Concat_84653805404632.

Reference computation: x is [70, 128, 512] f32; rows 0..19 are supports
(ns_all = n_class*n_support = 20), rows 20..69 are queries (nq_all = 50).
Output [1000, 128, 1024] where out[q*20+s] = concat(sup[s], qry[q], axis=-1).

Pure data movement (memory regime). Sharding: the (query, support) pair grid
[50 x 20] is split as (2 query-halves) x (4 support-fifths) -> 8 cores, each
producing exactly 125 output rows (64 MB) with an identical SPMD access
pattern.

Per core: the 5 support tiles are DMA-loaded directly into the sup columns of
two interleaved "image" buffers in SBUF; the VectorEngine broadcasts each
query tile into the qry columns (SBUF engine ports are separate from the DMA
AXI ports, so this overlaps the writes for free); each query then leaves as
ONE 2.62 MB write DMA whose descriptors are full 4 KB rows — the largest
descriptor this output layout allows, which keeps all 16 SDMA engines at
their ~25 B/ns per-descriptor rate (~400-420 GB/s per core, the 16-engine
descriptor-processing ceiling). Writes double-buffer against the DVE copies;
the load order (sup image 0, first query chunk, sup image 1, remaining
chunks) lets the first write start as early as possible.

Measured on 8 trn2 cores: 195-221 us NEFF exec depending on device state
(the chip oscillates between a ~420 GB/s/core and a ~350 GB/s/core HBM/DMA
regime run-to-run), rel err 0. Within a fixed state this layout beat the
per-query-load variant by ~4% by moving 1.3 MB fewer load bytes per core.
"""

import os
import sys

import numpy as np

for _p in ("/opt/trn_rl_repo", "/root/.axon_site/_ro/trn_rl_repo"):
    if os.path.isdir(_p) and _p not in sys.path:
        sys.path.insert(0, _p)

import concourse.bass as bass
import concourse.mybir as mybir
from concourse.bass_utils import run_bass_kernel_spmd

NS_ALL = 20  # n_class * n_support
NQ_ALL = 50  # n_class * n_query
D = 128
F = 512
QH = 25  # queries per core  (NQ_ALL / 2)
SF = 5  # supports per core (NS_ALL / 4)
QCH = 5  # query tiles per load chunk
N_CORES = 8

_NC_CACHE = None


def _build_nc():
    nc = bass.Bass()
    sup = nc.declare_dram_parameter("sup", [D, SF, F], mybir.dt.float32, isOutput=False)
    qry = nc.declare_dram_parameter("qry", [D, QH, F], mybir.dt.float32, isOutput=False)
    out = nc.declare_dram_parameter(
        "out", [D, QH * SF, 2 * F], mybir.dt.float32, isOutput=True
    )

    with (
        nc.sbuf_tensor([D, QH * F], mybir.dt.float32) as qry_t,
        nc.sbuf_tensor([D, SF * F], mybir.dt.float32) as sup_t,
        nc.sbuf_tensor([D, SF * 2 * F], mybir.dt.float32) as img0,
        nc.sbuf_tensor([D, SF * 2 * F], mybir.dt.float32) as img1,
        nc.semaphore("sup_sem") as sup_sem,
        nc.semaphore("qry_sem0") as qry_sem0,
        nc.semaphore("qry_sem1") as qry_sem1,
        nc.semaphore("qry_sem2") as qry_sem2,
        nc.semaphore("qry_sem3") as qry_sem3,
        nc.semaphore("qry_sem4") as qry_sem4,
        nc.semaphore("dve_sem") as dve_sem,
        nc.semaphore("out_sem0") as out_sem0,
        nc.semaphore("out_sem1") as out_sem1,
        nc.Block() as block,
    ):
        imgs = [img0, img1]
        qry_sems = [qry_sem0, qry_sem1, qry_sem2, qry_sem3, qry_sem4]
        out_sems = [out_sem0, out_sem1]

        def img_view(b):
            return imgs[b][:].rearrange("p (s f2) -> p s f2", f2=2 * F)

        @block.sync
        def _(sync):
            sync.dma_start(sup_t[:], sup[:]).then_inc(sup_sem, 16)
            for c in range(QH // QCH):
                sync.dma_start(
                    qry_t[:, QCH * F * c : QCH * F * (c + 1)],
                    qry[:, QCH * c : QCH * (c + 1), :],
                ).then_inc(qry_sems[c], 16)

        @block.vector
        def _(vector):
            sup_v = sup_t[:].rearrange("p (s f) -> p s f", f=F)
            vector.wait_ge(sup_sem, 16)
            vector.tensor_copy(img_view(0)[:, :, 0:F], sup_v).then_inc(dve_sem, 1)

            def qcopy(q):
                vector.wait_ge(qry_sems[q // QCH], 16)
                if q >= 2:
                    vector.wait_ge(out_sems[q % 2], 16 * (q // 2))
                dst = img_view(q % 2)[:, :, F : 2 * F]
                src = (
                    qry_t[:, F * q : F * (q + 1)]
                    .unsqueeze(1)
                    .broadcast_to([D, SF, F])
                )
                vector.tensor_copy(dst, src).then_inc(dve_sem, 1)

            qcopy(0)
            vector.tensor_copy(img_view(1)[:, :, 0:F], sup_v).then_inc(dve_sem, 1)
            for q in range(1, QH):
                qcopy(q)

        @block.scalar
        def _(scalar):
            for q in range(QH):
                scalar.wait_ge(dve_sem, 2 if q == 0 else q + 3)
                # p-major dst: rows 5q..5q+5 of every partition, contiguous
                # 20KB per partition; 4KB descriptors via max_dma_last_dim
                dst = out[:, SF * q : SF * (q + 1), :]
                scalar.dma_start(
                    dst, imgs[q % 2][:], max_dma_last_dim=4096
                ).then_inc(out_sems[q % 2], 16)
            scalar.wait_ge(out_sem0, 16 * ((QH + 1) // 2))
            scalar.wait_ge(out_sem1, 16 * (QH // 2))

    return nc


def _get_nc():
    global _NC_CACHE
    if _NC_CACHE is None:
        _NC_CACHE = _build_nc()
    return _NC_CACHE


def kernel(**inputs) -> np.ndarray:
    x = np.ascontiguousarray(np.asarray(inputs["x"], dtype=np.float32))
    assert x.shape == (NS_ALL + NQ_ALL, D, F), x.shape

    sup_all = x[:NS_ALL]
    qry_all = x[NS_ALL:]

    in_maps = []
    for k in range(N_CORES):
        h, f = divmod(k, 4)
        in_maps.append(
            {
                # transposed to [D, n, F] so load DMAs are contiguous on both
                # sides (4KB descriptors via max_dma_last_dim)
                "sup": np.ascontiguousarray(
                    sup_all[SF * f : SF * (f + 1)].transpose(1, 0, 2)
                ),
                "qry": np.ascontiguousarray(
                    qry_all[QH * h : QH * (h + 1)].transpose(1, 0, 2)
                ),
            }
        )

    nc = _get_nc()
    res = run_bass_kernel_spmd(nc, in_maps, core_ids=list(range(N_CORES)))

    full = np.empty((NQ_ALL, NS_ALL, D, 2 * F), dtype=np.float32)
    for k in range(N_CORES):
        h, f = divmod(k, 4)
        out_k = (
            np.asarray(res.results[k]["out"])
            .transpose(1, 0, 2)
            .reshape(QH, SF, D, 2 * F)
        )
        full[QH * h : QH * (h + 1), SF * f : SF * (f + 1)] = out_k
    return full.reshape(NQ_ALL * NS_ALL, D, 2 * F)


# revision 18
# speedup vs baseline: 1.0641x; 1.0614x over previous
"""Trainium2 Bass kernel for nn_Concat_84653805404632.

Reference computation: x is [70, 128, 512] f32; rows 0..19 are supports
(ns_all = n_class*n_support = 20), rows 20..69 are queries (nq_all = 50).
Output [1000, 128, 1024] where out[q*20+s] = concat(sup[s], qry[q], axis=-1).

Pure data movement (memory regime). Sharding: the (query, support) pair grid
[50 x 20] is split as (2 query-halves) x (4 support-fifths) -> 8 cores, each
producing exactly 125 output rows (64 MB) with an identical SPMD access
pattern.

Per core: the 5 support tiles are DMA-loaded directly into the sup columns of
two interleaved "image" buffers in SBUF; the VectorEngine broadcasts each
query tile into the qry columns (SBUF engine ports are separate from the DMA
AXI ports, so this overlaps the writes for free); each query then leaves as
ONE 2.62 MB write DMA whose descriptors are full 4 KB rows — the largest
descriptor this output layout allows, which keeps all 16 SDMA engines at
their ~25 B/ns per-descriptor rate (~400-420 GB/s per core, the 16-engine
descriptor-processing ceiling). Writes double-buffer against the DVE copies;
the load order (sup image 0, first query chunk, sup image 1, remaining
chunks) lets the first write start as early as possible.

Measured on 8 trn2 cores: 195-221 us NEFF exec depending on device state
(the chip oscillates between a ~420 GB/s/core and a ~350 GB/s/core HBM/DMA
regime run-to-run), rel err 0. Within a fixed state this layout beat the
per-query-load variant by ~4% by moving 1.3 MB fewer load bytes per core.
"""

import os
import sys

import numpy as np

for _p in ("/opt/trn_rl_repo", "/root/.axon_site/_ro/trn_rl_repo"):
    if os.path.isdir(_p) and _p not in sys.path:
        sys.path.insert(0, _p)

import concourse.bass as bass
import concourse.mybir as mybir
from concourse.bass_utils import run_bass_kernel_spmd

NS_ALL = 20  # n_class * n_support
NQ_ALL = 50  # n_class * n_query
D = 128
F = 512
QH = 25  # queries per core  (NQ_ALL / 2)
SF = 5  # supports per core (NS_ALL / 4)
QCH = 5  # query tiles per load chunk
N_CORES = 8

_NC_CACHE = None


def _build_nc():
    nc = bass.Bass()
    # host passes transposed shards: sup_r [D, SF, F], qry_r [D, QH, F]
    sup = nc.declare_dram_parameter("sup", [D, SF, F], mybir.dt.float32, isOutput=False)
    qry = nc.declare_dram_parameter("qry", [D, QH, F], mybir.dt.float32, isOutput=False)
    out = nc.declare_dram_parameter(
        "out", [QH * SF, D, 2 * F], mybir.dt.float32, isOutput=True
    )

    with (
        nc.sbuf_tensor([D, QH * F], mybir.dt.float32) as qry_t,
        nc.sbuf_tensor([D, SF * F], mybir.dt.float32) as sup_t,
        nc.sbuf_tensor([D, SF * 2 * F], mybir.dt.float32) as img0,
        nc.sbuf_tensor([D, SF * 2 * F], mybir.dt.float32) as img1,
        nc.semaphore("sup_sem") as sup_sem,
        nc.semaphore("qry_sem0") as qry_sem0,
        nc.semaphore("qry_sem1") as qry_sem1,
        nc.semaphore("qry_sem2") as qry_sem2,
        nc.semaphore("qry_sem3") as qry_sem3,
        nc.semaphore("qry_sem4") as qry_sem4,
        nc.semaphore("dve_sem") as dve_sem,
        nc.semaphore("out_sem0") as out_sem0,
        nc.semaphore("out_sem1") as out_sem1,
        nc.Block() as block,
    ):
        imgs = [img0, img1]
        qry_sems = [qry_sem0, qry_sem1, qry_sem2, qry_sem3, qry_sem4]
        out_sems = [out_sem0, out_sem1]

        def img_view(b):
            return imgs[b][:].rearrange("p (s f2) -> p s f2", f2=2 * F)

        @block.sync
        def _(sync):
            # all loads contiguous on both sides -> >=4KB descriptors
            sync.dma_start(sup_t[:], sup[:]).then_inc(sup_sem, 16)
            for c in range(QH // QCH):
                sync.dma_start(
                    qry_t[:, QCH * F * c : QCH * F * (c + 1)],
                    qry[:, QCH * c : QCH * (c + 1), :],
                ).then_inc(qry_sems[c], 16)

        @block.vector
        def _(vector):
            sup_v = sup_t[:].rearrange("p (s f) -> p s f", f=F)
            # op order: mirror img0, copy q0, mirror img1, copy q1, copies q2+
            # (write q waits dve_sem >= q + 3 for q >= 1; write 0 waits >= 2)
            vector.wait_ge(sup_sem, 16)
            vector.tensor_copy(img_view(0)[:, :, 0:F], sup_v).then_inc(dve_sem, 1)

            def qcopy(q):
                vector.wait_ge(qry_sems[q // QCH], 16)
                if q >= 2:
                    vector.wait_ge(out_sems[q % 2], 16 * (q // 2))
                dst = img_view(q % 2)[:, :, F : 2 * F]
                src = (
                    qry_t[:, F * q : F * (q + 1)]
                    .unsqueeze(1)
                    .broadcast_to([D, SF, F])
                )
                vector.tensor_copy(dst, src).then_inc(dve_sem, 1)

            qcopy(0)
            vector.tensor_copy(img_view(1)[:, :, 0:F], sup_v).then_inc(dve_sem, 1)
            for q in range(1, QH):
                qcopy(q)

        @block.scalar
        def _(scalar):
            for q in range(QH):
                scalar.wait_ge(dve_sem, 2 if q == 0 else q + 3)
                dst = out[SF * q : SF * (q + 1), :, :].transpose([1, 0, 2])
                scalar.dma_start(dst, imgs[q % 2][:]).then_inc(out_sems[q % 2], 16)
            scalar.wait_ge(out_sem0, 16 * ((QH + 1) // 2))
            scalar.wait_ge(out_sem1, 16 * (QH // 2))

    return nc


def _get_nc():
    global _NC_CACHE
    if _NC_CACHE is None:
        _NC_CACHE = _build_nc()
    return _NC_CACHE


def kernel(**inputs) -> np.ndarray:
    x = np.ascontiguousarray(np.asarray(inputs["x"], dtype=np.float32))
    assert x.shape == (NS_ALL + NQ_ALL, D, F), x.shape

    sup_all = x[:NS_ALL]
    qry_all = x[NS_ALL:]

    in_maps = []
    for k in range(N_CORES):
        h, f = divmod(k, 4)
        in_maps.append(
            {
                # transposed to [D, n, F] so load DMAs are contiguous on both
                # sides (4KB descriptors via max_dma_last_dim)
                "sup": np.ascontiguousarray(
                    sup_all[SF * f : SF * (f + 1)].transpose(1, 0, 2)
                ),
                "qry": np.ascontiguousarray(
                    qry_all[QH * h : QH * (h + 1)].transpose(1, 0, 2)
                ),
            }
        )

    nc = _get_nc()
    res = run_bass_kernel_spmd(nc, in_maps, core_ids=list(range(N_CORES)))

    full = np.empty((NQ_ALL, NS_ALL, D, 2 * F), dtype=np.float32)
    for k in range(N_CORES):
        h, f = divmod(k, 4)
        out_k = np.asarray(res.results[k]["out"]).reshape(QH, SF, D, 2 * F)
        full[QH * h : QH * (h + 1), SF * f : SF * (f + 1)] = out_k
    return full.reshape(NQ_ALL * NS_ALL, D, 2 * F)
